# revision 3
# baseline (speedup 1.0000x reference)
"""Llama-style GQA attention (T=2048, H=4096, 32 q heads, 8 kv heads, d=128)
on 8 Trainium2 NeuronCores.

Sharding (tensor-parallel over heads, matching GQA groups):
  core c owns q heads 4c..4c+3 (512 cols of wq) and kv head c (128 cols of
  wk/wv).  Everything on-device is kept feature-major ("transposed"):
  x is fed as xT [H, T]; projections compute qT/kT/vT directly with the
  weight slices as the matmul stationary operand; attention produces
  scoresT [k_tok, q_tok] tiles so the PV matmul needs no transposes; the
  per-core attention output attnT_c [512, T] is AllGathered into the full
  attnT [H, T], from which each core computes its 512-column slice of the
  o_proj output as outT_c [512, T] = wo[:, cols].T @ attnT.  The host
  transposes/concats the 8 outT_c shards into the final [T, H] output.

Matmuls run in bf16 (fp32 PSUM accumulation); softmax runs unnormalized
exp (scores are O(10) here so no max-subtraction is needed) with the
row-sum Z accumulated on the vector engine and applied as 1/Z at PSUM
eviction via a gpsimd partition-broadcast.
"""

import math

import ml_dtypes
import numpy as np

import concourse.bacc as bacc
import concourse.mybir as mybir
import concourse.tile as tile
from concourse.bass_utils import run_bass_kernel_spmd

T, H, N_HEADS, N_KV, HEAD_DIM = 2048, 4096, 32, 8, 128
ROPE_BASE = 10000.0
N_CORES = 8
HPC = N_HEADS // N_CORES          # q heads per core (4)
QD = HPC * HEAD_DIM               # per-core q/attn width (512)
P = 128
NT = T // 512                     # token ranges of 512 (4)
KTILES = H // P                   # hidden contraction tiles (32)
TTILES = T // P                   # token tiles of 128 (16)
SCALE = 1.0 / math.sqrt(HEAD_DIM)

BF = mybir.dt.bfloat16
F32 = mybir.dt.float32
AFT = mybir.ActivationFunctionType
MULT = mybir.AluOpType.mult
ADD = mybir.AluOpType.add

_CACHE = {}


def _build():
    nc = bacc.Bacc("TRN2", target_bir_lowering=False, debug=False,
                   num_devices=N_CORES)

    xt_d = nc.dram_tensor("xt", [H, T], BF, kind="ExternalInput")
    wq_d = nc.dram_tensor("wqs", [H, QD], BF, kind="ExternalInput")
    wk_d = nc.dram_tensor("wks", [H, HEAD_DIM], BF, kind="ExternalInput")
    wv_d = nc.dram_tensor("wvs", [H, HEAD_DIM], BF, kind="ExternalInput")
    wo_d = nc.dram_tensor("wos", [H, QD], BF, kind="ExternalInput")
    cost_d = nc.dram_tensor("cost", [P, T], F32, kind="ExternalInput")
    sint_d = nc.dram_tensor("sint", [P, T], F32, kind="ExternalInput")
    mask_d = nc.dram_tensor("maskp", [P, 4 * 512], BF, kind="ExternalInput")
    ones_d = nc.dram_tensor("onesc", [P, 1], F32, kind="ExternalInput")
    ident_d = nc.dram_tensor("ident", [P, P], BF, kind="ExternalInput")
    out_d = nc.dram_tensor("out", [QD, T], F32, kind="ExternalOutput")

    with tile.TileContext(nc) as tc:
        with (
            tc.tile_pool(name="const", bufs=1) as const,
            tc.tile_pool(name="acts", bufs=1) as acts,
            tc.tile_pool(name="dram", bufs=1, space="DRAM") as dram,
        ):
            cost = const.tile([P, T], F32)
            sint = const.tile([P, T], F32)
            maskp = const.tile([P, 4 * 512], BF)
            ones = const.tile([P, 1], F32)
            ident = const.tile([P, P], BF)
            for t, d in ((cost, cost_d), (sint, sint_d), (maskp, mask_d),
                         (ones, ones_d), (ident, ident_d)):
                nc.sync.dma_start(t[:], d[:])

            wk_sb = const.tile([P, KTILES, HEAD_DIM], BF)
            wv_sb = const.tile([P, KTILES, HEAD_DIM], BF)
            wq_sb = const.tile([P, KTILES, QD], BF)
            wo_sb = const.tile([P, KTILES, QD], BF)
            for kt in range(KTILES):
                rows = slice(kt * P, (kt + 1) * P)
                nc.sync.dma_start(wk_sb[:, kt, :], wk_d[rows, :])
                nc.sync.dma_start(wv_sb[:, kt, :], wv_d[rows, :])
                nc.sync.dma_start(wq_sb[:, kt, :], wq_d[rows, :])
            for kt in range(KTILES):
                rows = slice(kt * P, (kt + 1) * P)
                nc.sync.dma_start(wo_sb[:, kt, :], wo_d[rows, :])

            # persistent per-core activations
            qr_sb = acts.tile([P, HPC, T], BF)     # rope'd qT per head
            kr_sb = acts.tile([P, T], BF)          # rope'd kT
            v_sb = acts.tile([P, TTILES, P], BF)   # v, token-major tiles

            attn_self = dram.tile([QD, T], BF)
            attn_all = dram.tile([H, T], BF, addr_space="Shared")

            # ---------- phase 1: q/k/v projections + rope + v transpose ----
            with (
                tc.tile_pool(name="xin", bufs=3) as xin,
                tc.tile_pool(name="rope", bufs=2) as rope,
                tc.tile_pool(name="vwork", bufs=2) as vwork,
                tc.tile_pool(name="pproj", bufs=1, space="PSUM") as pproj,
                tc.tile_pool(name="ptr", bufs=2, space="PSUM") as ptr,
            ):
                def do_rope(psum_in, dst_ap, ts_):
                    t1 = rope.tile([P, 512], F32, tag="t1")
                    t2 = rope.tile([P, 512], F32, tag="t2")
                    nc.vector.tensor_tensor(t1[:], psum_in[:], cost[:, ts_], MULT)
                    nc.vector.tensor_tensor(t2[0:64, :], psum_in[64:128, :],
                                            sint[0:64, ts_], MULT)
                    nc.vector.tensor_tensor(t2[64:128, :], psum_in[0:64, :],
                                            sint[64:128, ts_], MULT)
                    nc.vector.tensor_tensor(dst_ap, t1[:], t2[:], ADD)

                for tq in range(NT):
                    ts_ = slice(tq * 512, (tq + 1) * 512)
                    kp = pproj.tile([P, 512], F32, tag="kp")
                    vp = pproj.tile([P, 512], F32, tag="vp")
                    qps = [pproj.tile([P, 512], F32, tag=f"qp{h}",
                                      name=f"qp{h}_{tq}")
                           for h in range(HPC)]
                    for kt in range(KTILES):
                        xtile = xin.tile([P, 512], BF)
                        nc.sync.dma_start(
                            xtile[:], xt_d[kt * P:(kt + 1) * P, ts_])
                        st, sp = (kt == 0), (kt == KTILES - 1)
                        nc.tensor.matmul(kp[:], wk_sb[:, kt, :], xtile[:],
                                         start=st, stop=sp)
                        nc.tensor.matmul(vp[:], wv_sb[:, kt, :], xtile[:],
                                         start=st, stop=sp)
                        for h in range(HPC):
                            nc.tensor.matmul(
                                qps[h][:],
                                wq_sb[:, kt, h * P:(h + 1) * P], xtile[:],
                                start=st, stop=sp)
                    do_rope(kp, kr_sb[:, ts_], ts_)
                    for h in range(HPC):
                        do_rope(qps[h], qr_sb[:, h, ts_], ts_)
                    vt = vwork.tile([P, 512], BF)
                    nc.scalar.copy(vt[:], vp[:])
                    for j in range(4):
                        tp = ptr.tile([P, P], BF)
                        nc.tensor.transpose(
                            tp[:], vt[:, j * P:(j + 1) * P], ident[:])
                        nc.vector.tensor_copy(v_sb[:, tq * 4 + j, :], tp[:])

            # ---------- phase 2: causal attention per head ----------
            with (
                tc.tile_pool(name="spool", bufs=3, space="PSUM") as spool,
                tc.tile_pool(name="opool", bufs=2, space="PSUM") as opool,
                tc.tile_pool(name="zps", bufs=2, space="PSUM") as zps,
                tc.tile_pool(name="expool", bufs=4) as expool,
                tc.tile_pool(name="zwork", bufs=2) as zwork,
                tc.tile_pool(name="atout", bufs=2) as atout,
            ):
                for h in range(HPC):
                    for qr in range(NT):
                        qs = slice(qr * 512, (qr + 1) * 512)
                        KT = 4 * (qr + 1)
                        op_ps = opool.tile([P, 512], F32, tag="op")
                        zacc = zwork.tile([P, 512], F32, tag="zacc")
                        for kt in range(KT):
                            sp = spool.tile([P, 512], F32, tag="sp")
                            nc.tensor.matmul(
                                sp[:], kr_sb[:, kt * P:(kt + 1) * P],
                                qr_sb[:, h, qs], start=True, stop=True)
                            ex = expool.tile([P, 512], BF, tag="ex")
                            nc.scalar.activation(ex[:], sp[:], AFT.Exp,
                                                 scale=SCALE)
                            j = kt - 4 * qr
                            if j >= 0:  # diagonal-straddling tile
                                nc.vector.tensor_tensor(
                                    ex[:], ex[:],
                                    maskp[:, j * 512:(j + 1) * 512], MULT)
                            if kt == 0:
                                nc.vector.tensor_copy(zacc[:], ex[:])
                            else:
                                nc.vector.tensor_tensor(
                                    zacc[:], zacc[:], ex[:], ADD)
                            nc.tensor.matmul(op_ps[:], v_sb[:, kt, :], ex[:],
                                             start=(kt == 0),
                                             stop=(kt == KT - 1))
                        zp = zps.tile([1, 512], F32, tag="zp")
                        nc.tensor.matmul(zp[:], ones[:], zacc[:],
                                         start=True, stop=True)
                        zr = zwork.tile([1, 512], F32, tag="zr")
                        nc.vector.reciprocal(zr[:], zp[:])
                        zb = zwork.tile([P, 512], F32, tag="zb")
                        nc.gpsimd.partition_broadcast(zb[:], zr[0:1, :])
                        at_sb = atout.tile([P, 512], BF)
                        nc.vector.tensor_tensor(at_sb[:], op_ps[:], zb[:],
                                                MULT)
                        nc.sync.dma_start(
                            attn_self[h * P:(h + 1) * P, qs], at_sb[:])

            # ---------- phase 3: AllGather attnT across cores ----------
            nc.gpsimd.collective_compute(
                "AllGather",
                mybir.AluOpType.bypass,
                replica_groups=[list(range(N_CORES))],
                ins=[attn_self.opt()],
                outs=[attn_all.opt()],
            )

            # ---------- phase 4: o_proj slice outT_c = wo_c.T @ attnT ------
            with (
                tc.tile_pool(name="rin", bufs=3) as rin,
                tc.tile_pool(name="oout", bufs=3) as oout,
                tc.tile_pool(name="pout", bufs=2, space="PSUM") as pout,
            ):
                for n in range(NT):
                    ns = slice(n * 512, (n + 1) * 512)
                    ops = [pout.tile([P, 512], F32, tag=f"om{m}",
                                     name=f"om{m}_{n}")
                           for m in range(4)]
                    for kt in range(KTILES):
                        rt = rin.tile([P, 512], BF)
                        nc.sync.dma_start(
                            rt[:], attn_all[kt * P:(kt + 1) * P, ns])
                        for m in range(4):
                            nc.tensor.matmul(
                                ops[m][:], wo_sb[:, kt, m * P:(m + 1) * P],
                                rt[:], start=(kt == 0), stop=(kt == KTILES - 1))
                    for m in range(4):
                        ot = oout.tile([P, 512], F32)
                        nc.vector.tensor_copy(ot[:], ops[m][:])
                        nc.sync.dma_start(out_d[m * P:(m + 1) * P, ns], ot[:])

    nc.compile()
    return nc


def _get_nc():
    if "nc" not in _CACHE:
        _CACHE["nc"] = _build()
    return _CACHE["nc"]


def _prep_inputs(x, positions, wq, wk, wv, wo):
    bf = ml_dtypes.bfloat16
    xt = np.ascontiguousarray(x.T.astype(bf))

    # rope tables, mirroring the reference (fp32 math)
    pos = np.clip(positions.astype(np.int64), 0, T - 1)
    inv_freq = (1.0 / (ROPE_BASE ** (np.arange(0, HEAD_DIM, 2,
                                               dtype=np.float32) / HEAD_DIM)))
    freqs = pos[:, None].astype(np.float32) * inv_freq[None, :]
    cos_b = np.repeat(np.cos(freqs), 2, axis=1)    # [T, 128]
    sin_b = np.repeat(np.sin(freqs), 2, axis=1)
    cost = np.ascontiguousarray(cos_b.T.astype(np.float32))   # [128, T]
    sint_raw = sin_b.T.astype(np.float32)
    sints = np.concatenate([-sint_raw[:64], sint_raw[64:]], axis=0)
    sints = np.ascontiguousarray(sints)

    # causal mask patterns for the 4 diagonal-straddling kt tiles of a
    # 512-wide q range: maskp[kl, 512*j + qf] = (qf >= 128*j + kl)
    qf = np.arange(512)
    kl = np.arange(P)
    maskp = np.concatenate(
        [(qf[None, :] >= (P * j + kl[:, None])).astype(bf) for j in range(4)],
        axis=1)
    maskp = np.ascontiguousarray(maskp)

    onesc = np.ones((P, 1), dtype=np.float32)
    ident = np.eye(P, dtype=bf)

    in_maps = []
    for c in range(N_CORES):
        qcols = slice(c * QD, (c + 1) * QD)
        kvcols = slice(c * HEAD_DIM, (c + 1) * HEAD_DIM)
        in_maps.append({
            "xt": xt,
            "wqs": np.ascontiguousarray(wq[:, qcols].astype(bf)),
            "wks": np.ascontiguousarray(wk[:, kvcols].astype(bf)),
            "wvs": np.ascontiguousarray(wv[:, kvcols].astype(bf)),
            "wos": np.ascontiguousarray(wo[:, qcols].astype(bf)),
            "cost": cost,
            "sint": sints,
            "maskp": maskp,
            "onesc": onesc,
            "ident": ident,
        })
    return in_maps


def _run(x, positions, wq, wk, wv, wo, **run_kwargs):
    nc = _get_nc()
    in_maps = _prep_inputs(np.asarray(x), np.asarray(positions),
                           np.asarray(wq), np.asarray(wk), np.asarray(wv),
                           np.asarray(wo))
    res = run_bass_kernel_spmd(nc, in_maps, core_ids=list(range(N_CORES)),
                               **run_kwargs)
    out = np.concatenate(
        [np.asarray(res.results[c]["out"]).T for c in range(N_CORES)], axis=1)
    return out.astype(np.float32), res


def kernel(x, positions, wq, wk, wv, wo):
    out, _ = _run(x, positions, wq, wk, wv, wo)
    return out


# revision 6
# speedup vs baseline: 1.2431x; 1.2431x over previous
"""Llama-style GQA attention (T=2048, H=4096, 32 q heads, 8 kv heads, d=128)
on 8 Trainium2 NeuronCores.

Sharding (tensor-parallel over heads, matching GQA groups):
  core c owns q heads 4c..4c+3 (512 cols of wq) and kv head c (128 cols of
  wk/wv).  Everything on-device is kept feature-major ("transposed"):
  x is fed as xT [H, T]; projections compute qT/kT/vT directly with the
  weight slices as the matmul stationary operand; attention produces
  scoresT [k_tok, q_tok] tiles so the PV matmul needs no transposes; the
  per-core attention output attnT_c [512, T] is AllGathered (in two token
  halves, so o_proj can start while attention finishes) into the full
  attnT [H, T], from which each core computes its 512-column slice of the
  o_proj output as outT_c [512, T] = wo[:, cols].T @ attnT.  The host
  transposes/concats the 8 outT_c shards into the final [T, H] output.

Matmuls run in bf16 (fp32 PSUM accumulation); softmax runs unnormalized
exp (scores are O(10) here, no max-subtraction needed).  The softmax
denominator Z is accumulated on the PE via a ones-column matmul over the
same bf16 exp tiles the PV matmul consumes, and 1/Z is applied at PSUM
eviction via reciprocal_approx_fast + gpsimd partition-broadcast.

Weights/x are fed host-pre-tiled as [128, KTILES, width] so each weight
lands in SBUF with a single large contiguous DMA.
"""

import math

import ml_dtypes
import numpy as np

import concourse.bacc as bacc
import concourse.mybir as mybir
import concourse.tile as tile
from concourse.bass_utils import run_bass_kernel_spmd

T, H, N_HEADS, N_KV, HEAD_DIM = 2048, 4096, 32, 8, 128
ROPE_BASE = 10000.0
N_CORES = 8
HPC = N_HEADS // N_CORES          # q heads per core (4)
QD = HPC * HEAD_DIM               # per-core q/attn width (512)
P = 128
NT = T // 512                     # token ranges of 512 (4)
KTILES = H // P                   # hidden contraction tiles (32)
TTILES = T // P                   # token tiles of 128 (16)
SCALE = 1.0 / math.sqrt(HEAD_DIM)

BF = mybir.dt.bfloat16
F32 = mybir.dt.float32
AFT = mybir.ActivationFunctionType
MULT = mybir.AluOpType.mult
ADD = mybir.AluOpType.add

_CACHE = {}


def _build():
    nc = bacc.Bacc("TRN2", target_bir_lowering=False, debug=False,
                   num_devices=N_CORES)

    xt_d = nc.dram_tensor("xt", [P, KTILES, T], BF, kind="ExternalInput")
    wq_d = nc.dram_tensor("wqs", [P, KTILES, QD], BF, kind="ExternalInput")
    wk_d = nc.dram_tensor("wks", [P, KTILES, HEAD_DIM], BF, kind="ExternalInput")
    wv_d = nc.dram_tensor("wvs", [P, KTILES, HEAD_DIM], BF, kind="ExternalInput")
    wo_d = nc.dram_tensor("wos", [P, KTILES, QD], BF, kind="ExternalInput")
    cost_d = nc.dram_tensor("cost", [P, T], F32, kind="ExternalInput")
    sint_d = nc.dram_tensor("sint", [P, T], F32, kind="ExternalInput")
    mask_d = nc.dram_tensor("maskp", [P, 4 * 512], BF, kind="ExternalInput")
    ones_d = nc.dram_tensor("onesc", [P, 1], BF, kind="ExternalInput")
    ident_d = nc.dram_tensor("ident", [P, P], BF, kind="ExternalInput")
    out_d = nc.dram_tensor("out", [QD, T], F32, kind="ExternalOutput")

    with tile.TileContext(nc) as tc:
        with (
            tc.tile_pool(name="const", bufs=1) as const,
            tc.tile_pool(name="acts", bufs=1) as acts,
            tc.tile_pool(name="dram", bufs=1, space="DRAM") as dram,
        ):
            wk_sb = const.tile([P, KTILES, HEAD_DIM], BF)
            wv_sb = const.tile([P, KTILES, HEAD_DIM], BF)
            wq_sb = const.tile([P, KTILES, QD], BF)
            cost = const.tile([P, T], F32)
            sint = const.tile([P, T], F32)
            maskp = const.tile([P, 4 * 512], BF)
            ones = const.tile([P, 1], BF)
            ident = const.tile([P, P], BF)
            wo_sb = const.tile([P, KTILES, QD], BF)
            for t, d in ((wk_sb, wk_d), (wv_sb, wv_d), (wq_sb, wq_d),
                         (cost, cost_d), (sint, sint_d), (maskp, mask_d),
                         (ones, ones_d), (ident, ident_d), (wo_sb, wo_d)):
                nc.sync.dma_start(t[:], d[:])

            # persistent per-core activations (per-head q tiles so attention
            # on head h can start as soon as its projection pass finishes)
            qr_sbs = [acts.tile([P, T], BF, name=f"qr{h}") for h in range(HPC)]
            kr_sb = acts.tile([P, T], BF)
            v_sb = acts.tile([P, TTILES, P], BF)

            attn_self = [dram.tile([QD, 1024], BF, name=f"aself{i}")
                         for i in range(2)]
            attn_all = [dram.tile([H, 1024], BF, addr_space="Shared",
                                  name=f"aall{i}") for i in range(2)]

            # ---------- phase 1: q/k/v projections + rope + v transpose ----
            with (
                tc.tile_pool(name="xin", bufs=3) as xin,
                tc.tile_pool(name="rope", bufs=2) as rope,
                tc.tile_pool(name="vwork", bufs=2) as vwork,
            ):
                def do_rope(psum_in, dst_ap, ts_):
                    t1 = rope.tile([P, 512], F32, tag="t1", name="t1")
                    t2 = rope.tile([P, 512], F32, tag="t2", name="t2")
                    nc.vector.tensor_tensor(t1[:], psum_in, cost[:, ts_], MULT)
                    nc.vector.tensor_tensor(t2[0:64, :], psum_in[64:128, :],
                                            sint[0:64, ts_], MULT)
                    nc.vector.tensor_tensor(t2[64:128, :], psum_in[0:64, :],
                                            sint[64:128, ts_], MULT)
                    nc.vector.tensor_tensor(dst_ap, t1[:], t2[:], ADD)

                # pass 1: k + v over all tokens
                kv_ctx = tc.tile_pool(name="pkv", bufs=1, space="PSUM")
                pkv = kv_ctx.__enter__()
                kp = pkv.tile([P, NT, 512], F32, tag="kp", name="kp")
                vp = pkv.tile([P, NT, 512], F32, tag="vp", name="vp")
                for kt in range(KTILES):
                    xtile = xin.tile([P, T], BF, tag="x1", name="x1")
                    nc.sync.dma_start(xtile[:], xt_d[:, kt, :])
                    st, sp = (kt == 0), (kt == KTILES - 1)
                    for n in range(NT):
                        nc.tensor.matmul(kp[:, n, :], wk_sb[:, kt, :],
                                         xtile[:, n * 512:(n + 1) * 512],
                                         start=st, stop=sp)
                    for n in range(NT):
                        nc.tensor.matmul(vp[:, n, :], wv_sb[:, kt, :],
                                         xtile[:, n * 512:(n + 1) * 512],
                                         start=st, stop=sp)
                vt_all = vwork.tile([P, T], BF, name="vt_all")
                for n in range(NT):
                    ts_ = slice(n * 512, (n + 1) * 512)
                    do_rope(kp[:, n, :], kr_sb[:, ts_], ts_)
                    nc.scalar.copy(vt_all[:, ts_], vp[:, n, :])
                kv_ctx.__exit__(None, None, None)

                # v transposes (own small PSUM pool)
                with tc.tile_pool(name="ptr", bufs=2, space="PSUM") as ptr:
                    for tt in range(TTILES):
                        tp = ptr.tile([P, P], BF, name="tp")
                        nc.tensor.transpose(
                            tp[:], vt_all[:, tt * P:(tt + 1) * P], ident[:])
                        nc.vector.tensor_copy(v_sb[:, tt, :], tp[:])

                # passes 2a/2b: all 4 q heads over token halves
                with tc.tile_pool(name="pq", bufs=1, space="PSUM") as pq:
                    for th in range(2):
                        hs = slice(th * 1024, (th + 1) * 1024)
                        qps = [pq.tile([P, 2, 512], F32, tag=f"qp{h}",
                                       name=f"qp{h}_{th}")
                               for h in range(HPC)]
                        for kt in range(KTILES):
                            xtile = xin.tile([P, 1024], BF, tag="x2", name="x2")
                            nc.sync.dma_start(xtile[:], xt_d[:, kt, hs])
                            st, sp = (kt == 0), (kt == KTILES - 1)
                            for h in range(HPC):
                                for n in range(2):
                                    nc.tensor.matmul(
                                        qps[h][:, n, :],
                                        wq_sb[:, kt, h * P:(h + 1) * P],
                                        xtile[:, n * 512:(n + 1) * 512],
                                        start=st, stop=sp)
                        for h in range(HPC):
                            for n in range(2):
                                ts_ = slice(th * 1024 + n * 512,
                                            th * 1024 + (n + 1) * 512)
                                do_rope(qps[h][:, n, :], qr_sbs[h][:, ts_],
                                        ts_)

            # ---------- phase 2: causal attention, heads in pairs ----------
            with (
                tc.tile_pool(name="spool", bufs=3, space="PSUM") as spool,
                tc.tile_pool(name="opool", bufs=1, space="PSUM") as opool,
                tc.tile_pool(name="zpool", bufs=1, space="PSUM") as zpool,
                tc.tile_pool(name="expool", bufs=4) as expool,
                tc.tile_pool(name="zwork", bufs=2) as zwork,
                tc.tile_pool(name="atout", bufs=2) as atout,
            ):
                for qr in range(NT):
                    qs = slice(qr * 512, (qr + 1) * 512)
                    KT = 4 * (qr + 1)
                    for hp in range(2):
                        heads = (2 * hp, 2 * hp + 1)
                        ops = [opool.tile([P, 512], F32, tag=f"op{i}",
                                          name=f"op{i}_{qr}_{hp}")
                               for i in range(2)]
                        zps = [zpool.tile([1, 512], F32, tag=f"zp{i}",
                                          name=f"zp{i}_{qr}_{hp}")
                               for i in range(2)]
                        for kt in range(KT):
                            sps = []
                            for i, h in enumerate(heads):
                                sp = spool.tile([P, 512], F32, tag="sp",
                                                name=f"sp{qr}_{hp}_{kt}_{i}")
                                nc.tensor.matmul(
                                    sp[:], kr_sb[:, kt * P:(kt + 1) * P],
                                    qr_sbs[h][:, qs], start=True, stop=True)
                                sps.append(sp)
                            st, spf = (kt == 0), (kt == KT - 1)
                            for i, h in enumerate(heads):
                                ex = expool.tile([P, 512], BF, tag="ex",
                                                 name=f"ex{qr}_{hp}_{kt}_{i}")
                                nc.scalar.activation(ex[:], sps[i][:],
                                                     AFT.Exp, scale=SCALE)
                                j = kt - 4 * qr
                                if j >= 0:  # diagonal-straddling tile
                                    nc.vector.tensor_tensor(
                                        ex[:], ex[:],
                                        maskp[:, j * 512:(j + 1) * 512], MULT)
                                nc.tensor.matmul(ops[i][:], v_sb[:, kt, :],
                                                 ex[:], start=st, stop=spf)
                                nc.tensor.matmul(zps[i][:], ones[:], ex[:],
                                                 start=st, stop=spf)
                        for i, h in enumerate(heads):
                            zr = zwork.tile([1, 512], F32, tag="zr", name="zr")
                            nc.vector.reciprocal_approx_fast(zr[:], zps[i][:])
                            zb = zwork.tile([P, 512], F32, tag="zb", name="zb")
                            nc.gpsimd.partition_broadcast(zb[:], zr[0:1, :])
                            at_sb = atout.tile([P, 512], BF, name="at_sb")
                            nc.vector.tensor_tensor(at_sb[:], ops[i][:],
                                                    zb[:], MULT)
                            nc.sync.dma_start(
                                attn_self[qr // 2][h * P:(h + 1) * P,
                                                   (qr % 2) * 512:
                                                   (qr % 2) * 512 + 512],
                                at_sb[:])

            # ---------- phase 3+4: AllGather halves + o_proj ----------
            with (
                tc.tile_pool(name="rin", bufs=3) as rin,
                tc.tile_pool(name="oout", bufs=4) as oout,
                tc.tile_pool(name="pout", bufs=1, space="PSUM") as pout,
            ):
                for half in range(2):
                    nc.gpsimd.collective_compute(
                        "AllGather",
                        mybir.AluOpType.bypass,
                        replica_groups=[list(range(N_CORES))],
                        ins=[attn_self[half].opt()],
                        outs=[attn_all[half].opt()],
                    )
                    ops = [pout.tile([P, 2, 512], F32, tag=f"om{m}",
                                     name=f"om{m}_{half}")
                           for m in range(4)]
                    for kt in range(KTILES):
                        rt = rin.tile([P, 1024], BF, name="rt")
                        nc.sync.dma_start(
                            rt[:], attn_all[half][kt * P:(kt + 1) * P, :])
                        st, spf = (kt == 0), (kt == KTILES - 1)
                        for m in range(4):
                            for n in range(2):
                                nc.tensor.matmul(
                                    ops[m][:, n, :],
                                    wo_sb[:, kt, m * P:(m + 1) * P],
                                    rt[:, n * 512:(n + 1) * 512],
                                    start=st, stop=spf)
                    for m in range(4):
                        for n in range(2):
                            ot = oout.tile([P, 512], F32, name="ot")
                            nc.vector.tensor_copy(ot[:], ops[m][:, n, :])
                            nc.sync.dma_start(
                                out_d[m * P:(m + 1) * P,
                                      half * 1024 + n * 512:
                                      half * 1024 + (n + 1) * 512],
                                ot[:])

    nc.compile()
    return nc


def _get_nc():
    if "nc" not in _CACHE:
        _CACHE["nc"] = _build()
    return _CACHE["nc"]


def _tile_km(w):
    """[H, W] -> [P, KTILES, W] with kt-major rows (host-side DMA layout)."""
    return np.ascontiguousarray(
        w.reshape(KTILES, P, w.shape[1]).transpose(1, 0, 2))


def _prep_inputs(x, positions, wq, wk, wv, wo):
    bf = ml_dtypes.bfloat16
    xt = _tile_km(np.ascontiguousarray(x.T).astype(bf))

    # rope tables, mirroring the reference (fp32 math)
    pos = np.clip(positions.astype(np.int64), 0, T - 1)
    inv_freq = (1.0 / (ROPE_BASE ** (np.arange(0, HEAD_DIM, 2,
                                               dtype=np.float32) / HEAD_DIM)))
    freqs = pos[:, None].astype(np.float32) * inv_freq[None, :]
    cos_b = np.repeat(np.cos(freqs), 2, axis=1)    # [T, 128]
    sin_b = np.repeat(np.sin(freqs), 2, axis=1)
    cost = np.ascontiguousarray(cos_b.T.astype(np.float32))   # [128, T]
    sint_raw = sin_b.T.astype(np.float32)
    sints = np.concatenate([-sint_raw[:64], sint_raw[64:]], axis=0)
    sints = np.ascontiguousarray(sints)

    # causal mask patterns for the 4 diagonal-straddling kt tiles of a
    # 512-wide q range: maskp[kl, 512*j + qf] = (qf >= 128*j + kl)
    qf = np.arange(512)
    kl = np.arange(P)
    maskp = np.concatenate(
        [(qf[None, :] >= (P * j + kl[:, None])).astype(bf) for j in range(4)],
        axis=1)
    maskp = np.ascontiguousarray(maskp)

    onesc = np.ones((P, 1), dtype=bf)
    ident = np.eye(P, dtype=bf)

    in_maps = []
    for c in range(N_CORES):
        qcols = slice(c * QD, (c + 1) * QD)
        kvcols = slice(c * HEAD_DIM, (c + 1) * HEAD_DIM)
        in_maps.append({
            "xt": xt,
            "wqs": _tile_km(wq[:, qcols].astype(bf)),
            "wks": _tile_km(wk[:, kvcols].astype(bf)),
            "wvs": _tile_km(wv[:, kvcols].astype(bf)),
            "wos": _tile_km(wo[:, qcols].astype(bf)),
            "cost": cost,
            "sint": sints,
            "maskp": maskp,
            "onesc": onesc,
            "ident": ident,
        })
    return in_maps


def _run(x, positions, wq, wk, wv, wo, **run_kwargs):
    nc = _get_nc()
    in_maps = _prep_inputs(np.asarray(x), np.asarray(positions),
                           np.asarray(wq), np.asarray(wk), np.asarray(wv),
                           np.asarray(wo))
    res = run_bass_kernel_spmd(nc, in_maps, core_ids=list(range(N_CORES)),
                               **run_kwargs)
    out = np.concatenate(
        [np.asarray(res.results[c]["out"]).T for c in range(N_CORES)], axis=1)
    return out.astype(np.float32), res


def kernel(x, positions, wq, wk, wv, wo):
    out, _ = _run(x, positions, wq, wk, wv, wo)
    return out
